# revision 1
# baseline (speedup 1.0000x reference)
"""Contrastive loss (NT-Xent) on 8 Trainium2 NeuronCores.

Row-parallel over the [2B, 2B] similarity matrix: core c computes rows
[c*1024, (c+1)*1024). Inputs are passed host-transposed ([D, 2B]) with the
column blocks rotated per core so the diagonal / positive blocks land at
fixed tile indices on every core (uniform SPMD program). Matmuls run in bf16
(full-rate PE path that engages the HAM clock un-throttle). Row-wise logsumexp uses the ACT
engine's fused accumulate; positives and the diagonal exclusion use
identity-mask reduces. Per-core partial sums are combined with a scalar
AllGather.
"""

import os
import sys

for _p in ("/opt/trn_rl_repo", "/root/.axon_site/_ro/trn_rl_repo"):
    if os.path.isdir(_p) and _p not in sys.path:
        sys.path.append(_p)

import numpy as np

B = 4096
D = 1024
TWO_B = 2 * B
TEMP = 0.07
N_CORES = 8
BLK = TWO_B // N_CORES  # 1024 rows per core
NT = TWO_B // 512  # 16 column tiles of 512
MT = BLK // 128  # 8 row tiles of 128
KT = D // 128  # 8 contraction chunks of 128

_cache = {}


def _build():
    import concourse.bass as bass
    import concourse.bacc as bacc
    import concourse.mybir as mybir
    from concourse.tile import TileContext

    f32 = mybir.dt.float32
    f32r = mybir.dt.float32r
    bf16 = mybir.dt.bfloat16
    AF = mybir.ActivationFunctionType
    ALU = mybir.AluOpType
    AX = mybir.AxisListType

    nc = bacc.Bacc(None, target_bir_lowering=False, debug=False)
    ft = nc.dram_tensor("ft", [D, TWO_B], f32, kind="ExternalInput")
    perm = nc.dram_tensor("perm", [8, 8], f32, kind="ExternalInput")
    ident = nc.dram_tensor("ident", [128, 128], f32, kind="ExternalInput")
    maskinv = nc.dram_tensor("maskinv", [128, 128], f32, kind="ExternalInput")
    loss = nc.dram_tensor("loss", [1, 1], f32, kind="ExternalOutput")

    with TileContext(nc) as tc:
        with (
            tc.tile_pool(name="own", bufs=KT) as pool_own,
            tc.tile_pool(name="big", bufs=1) as pool_big,
            tc.tile_pool(name="sq", bufs=2) as pool_sq,
            tc.tile_pool(name="rhs", bufs=10) as pool_rhs,
            tc.tile_pool(name="rhsr", bufs=10) as pool_rhsr,
            tc.tile_pool(name="exp", bufs=4) as pool_exp,
            tc.tile_pool(name="small", bufs=1) as pool_small,
            tc.tile_pool(name="rot", bufs=1) as pool_rot,
            tc.tile_pool(name="junk", bufs=2) as pool_junk,
            tc.tile_pool(name="psum", bufs=8, space="PSUM") as psum,
            tc.tile_pool(name="dram", bufs=4, space="DRAM") as dram,
        ):
            warm_in = dram.tile([1, 1], f32, name="warm_in")
            warm_out = dram.tile([8, 1], f32, name="warm_out")
            inv_cc_in = dram.tile([1, BLK], f32, name="inv_cc_in")
            inv_cc_out = dram.tile([8, BLK], f32, name="inv_cc_out")
            part_in = dram.tile([1, 1], f32, name="part_in")
            part_out = dram.tile([8, 1], f32, name="part_out")

            # --- collective-stack warmup: absorbs one-time ncfw/channel setup
            # concurrently with the prologue ---
            warm_sb = pool_small.tile([1, 1], f32, name="warm_sb", tag="warm_sb")
            nc.vector.memset(warm_sb[:], 0.0)
            nc.sync.dma_start(out=warm_in[:], in_=warm_sb[:])
            nc.gpsimd.collective_compute(
                "AllGather",
                mybir.AluOpType.bypass,
                ins=[warm_in.opt()],
                outs=[warm_out.opt()],
                replica_groups=[list(range(N_CORES))],
            )

            # --- constants ---
            ones_f = pool_small.tile([128, 1], f32, name="ones_f", tag="ones_f")
            nc.vector.memset(ones_f[:], 1.0)
            ones_r = pool_small.tile([128, 1], bf16, name="ones_r", tag="ones_r")
            nc.vector.tensor_copy(ones_r[:], ones_f[:])
            ones1_f = pool_small.tile([1, 128], f32, name="ones1_f", tag="ones1_f")
            nc.vector.memset(ones1_f[:], 1.0)
            ones1_r = pool_small.tile([1, 128], bf16, name="ones1_r", tag="ones1_r")
            nc.vector.tensor_copy(ones1_r[:], ones1_f[:])
            ident_sb = pool_small.tile([128, 128], f32, name="ident", tag="ident")
            nc.sync.dma_start(out=ident_sb[:], in_=ident[:])
            maskinv_sb = pool_small.tile([128, 128], f32, name="maskinv", tag="maskinv")
            nc.sync.dma_start(out=maskinv_sb[:], in_=maskinv[:])
            perm_f = pool_small.tile([8, 8], f32, name="perm_f", tag="perm_f")
            nc.sync.dma_start(out=perm_f[:], in_=perm[:])
            perm_r = pool_small.tile([8, 8], bf16, name="perm_r", tag="perm_r")
            nc.vector.tensor_copy(perm_r[:], perm_f[:])

            # --- own block: load + row norms ---
            own_raw = []
            for k in range(KT):
                t = pool_own.tile([128, BLK], f32, name="own_raw", tag="own_raw")
                nc.sync.dma_start(
                    out=t[:], in_=ft[k * 128 : (k + 1) * 128, 0:BLK]
                )
                own_raw.append(t)

            pss = [psum.tile([128, 512], f32, name="ps", tag="ps") for _ in range(2)]
            for k in range(KT):
                s = pool_sq.tile([128, BLK], bf16, name="sq", tag="sq")
                nc.vector.tensor_mul(s[:], own_raw[k][:], own_raw[k][:])
                for h in range(2):
                    nc.tensor.matmul(
                        pss[h][0:1, :],
                        ones_r[:],
                        s[:, h * 512 : (h + 1) * 512],
                        start=(k == 0),
                        stop=(k == KT - 1),
                    )
            nrm = pool_small.tile([1, BLK], f32, name="nrm", tag="nrm")
            for h in range(2):
                nc.scalar.activation(
                    nrm[:, h * 512 : (h + 1) * 512], pss[h][0:1, :], AF.Sqrt
                )
            inv_own = pool_small.tile([1, BLK], f32, name="inv_own", tag="inv_own")
            nc.vector.reciprocal(inv_own[:], nrm[:])

            # share inverse norms across cores
            nc.sync.dma_start(out=inv_cc_in[:], in_=inv_own[:])
            nc.gpsimd.collective_compute(
                "AllGather",
                mybir.AluOpType.bypass,
                ins=[inv_cc_in.opt()],
                outs=[inv_cc_out.opt()],
                replica_groups=[list(range(N_CORES))],
            )

            # binv[:, j*1024 + q] = inv norm of rotated column block j, col q,
            # replicated across all 128 partitions (PE rank-1 broadcast).
            binv = pool_big.tile([128, TWO_B], f32, name="binv", tag="binv")
            inv_own_r = pool_small.tile([1, BLK], bf16, name="inv_own_r", tag="inv_own_r")
            nc.vector.tensor_copy(inv_own_r[:], inv_own[:])
            for h in range(2):
                pb = psum.tile([128, 512], f32, name="ps", tag="ps")
                nc.tensor.matmul(
                    pb[:],
                    ones1_r[:],
                    inv_own_r[0:1, h * 512 : (h + 1) * 512],
                    start=True,
                    stop=True,
                )
                nc.vector.tensor_copy(binv[:, h * 512 : (h + 1) * 512], pb[:])

            # own block normalized: lhsT for all matmuls, rhs for n in {0, 1}
            own_nrm = []
            for k in range(KT):
                t = pool_own.tile([128, BLK], bf16, name="own_nrm", tag="own_nrm")
                nc.vector.tensor_mul(t[:], own_raw[k][:], binv[:, 0:BLK])
                own_nrm.append(t[:])

            # rotated inverse norms of the remote blocks
            g_inv = pool_small.tile([8, BLK], f32, name="g_inv", tag="g_inv")
            nc.sync.dma_start(out=g_inv[:], in_=inv_cc_out[:])
            g_inv_r = pool_small.tile([8, BLK], bf16, name="g_inv_r", tag="g_inv_r")
            nc.vector.tensor_copy(g_inv_r[:], g_inv[:])
            rot_r = pool_small.tile([8, BLK], bf16, name="rot_r", tag="rot_r")
            for h in range(2):
                pr = psum.tile([128, 512], f32, name="ps", tag="ps")
                nc.tensor.matmul(
                    pr[0:8, :],
                    perm_r[:],
                    g_inv_r[:, h * 512 : (h + 1) * 512],
                    start=True,
                    stop=True,
                )
                nc.vector.tensor_copy(rot_r[:, h * 512 : (h + 1) * 512], pr[0:8, :])
            # PE operands must start at partition 0/32/64 — move each rotated
            # row onto partition 0 before its rank-1 broadcast.
            for j in range(1, 8):
                rf = pool_rot.tile([1, BLK], bf16, name="rf", tag="rf")
                nc.sync.dma_start(out=rf[:], in_=rot_r[j : j + 1, :])
                for h in range(2):
                    pb = psum.tile([128, 512], f32, name="ps", tag="ps")
                    nc.tensor.matmul(
                        pb[:],
                        ones1_r[:],
                        rf[0:1, h * 512 : (h + 1) * 512],
                        start=True,
                        stop=True,
                    )
                    nc.vector.tensor_copy(
                        binv[:, j * BLK + h * 512 : j * BLK + (h + 1) * 512], pb[:]
                    )

            # --- accumulators ---
            rs_buf = pool_big.tile([128, MT * NT], f32, name="rs_buf", tag="rs_buf")
            pos_all = pool_small.tile([128, MT], f32, name="pos_all", tag="pos_all")
            nc.vector.memset(pos_all[:], 0.0)

            # --- main loop: one 512-wide column tile at a time ---
            n_limit = int(os.environ.get("CL_NT", NT))
            for n in range(n_limit):
                if n < 2:
                    rhs = [own_nrm[k][:, n * 512 : (n + 1) * 512] for k in range(KT)]
                else:
                    rhs = []
                    for k in range(KT):
                        raw = pool_rhs.tile([128, 512], f32, name="rhs_raw", tag="rhs_raw")
                        nc.sync.dma_start(
                            out=raw[:],
                            in_=ft[k * 128 : (k + 1) * 128, n * 512 : (n + 1) * 512],
                        )
                        r = pool_rhsr.tile([128, 512], bf16, name="rhs_r", tag="rhs_r")
                        nc.vector.tensor_mul(
                            r[:], raw[:], binv[:, n * 512 : (n + 1) * 512]
                        )
                        rhs.append(r[:])
                for m in range(MT):
                    ps = psum.tile([128, 512], f32, name="ps", tag="ps")
                    for k in range(KT):
                        nc.tensor.matmul(
                            ps[:],
                            own_nrm[k][:, m * 128 : (m + 1) * 128],
                            rhs[k],
                            start=(k == 0),
                            stop=(k == KT - 1),
                        )
                    sl = (m % 4) * 128
                    if n == 8 + m // 4:
                        # positives: diagonal of this 128x128 slab (raw sim)
                        junk = pool_junk.tile([128, 128], f32, name="junk", tag="junk")
                        nc.vector.tensor_mul(junk[:], ps[:, sl : sl + 128], ident_sb[:])
                        nc.vector.reduce_sum(
                            out=pos_all[:, m : m + 1], in_=junk[:], axis=AX.X
                        )
                    if n == m // 4:
                        # diagonal block: exp, zero the self-sim, reduce on DVE
                        e = pool_exp.tile([128, 512], f32, name="exp", tag="exp")
                        nc.scalar.activation(e[:], ps[:], AF.Exp, scale=1.0 / TEMP)
                        nc.vector.tensor_mul(
                            e[:, sl : sl + 128], e[:, sl : sl + 128], maskinv_sb[:]
                        )
                        nc.vector.reduce_sum(
                            out=rs_buf[:, m * NT + n : m * NT + n + 1],
                            in_=e[:],
                            axis=AX.X,
                        )
                    else:
                        e = pool_exp.tile([128, 512], f32, name="exp", tag="exp")
                        nc.scalar.activation(
                            e[:],
                            ps[:],
                            AF.Exp,
                            scale=1.0 / TEMP,
                            accum_out=rs_buf[:, m * NT + n : m * NT + n + 1],
                        )

            # --- logsumexp + loss ---
            rs_all = pool_small.tile([128, MT], f32, name="rs_all", tag="rs_all")
            for m in range(MT):
                nc.vector.reduce_sum(
                    out=rs_all[:, m : m + 1],
                    in_=rs_buf[:, m * NT : m * NT + n_limit],
                    axis=AX.X,
                )
            lse = pool_small.tile([128, MT], f32, name="lse", tag="lse")
            nc.scalar.activation(lse[:], rs_all[:], AF.Ln)
            poss = pool_small.tile([128, MT], f32, name="poss", tag="poss")
            nc.vector.tensor_scalar_mul(poss[:], pos_all[:], 1.0 / TEMP)
            diff = pool_small.tile([128, MT], f32, name="diff", tag="diff")
            nc.vector.tensor_sub(diff[:], lse[:], poss[:])
            dsum = pool_small.tile([128, 1], f32, name="dsum", tag="dsum")
            nc.vector.reduce_sum(out=dsum[:], in_=diff[:], axis=AX.X)
            pf = psum.tile([128, 512], f32, name="ps", tag="ps")
            nc.tensor.matmul(
                pf[0:1, 0:1], dsum[:], ones_f[:], start=True, stop=True
            )
            part_sb = pool_small.tile([1, 1], f32, name="part_sb", tag="part_sb")
            nc.vector.tensor_copy(part_sb[:], pf[0:1, 0:1])
            nc.sync.dma_start(out=part_in[:], in_=part_sb[:])
            nc.gpsimd.collective_compute(
                "AllGather",
                mybir.AluOpType.bypass,
                ins=[part_in.opt()],
                outs=[part_out.opt()],
                replica_groups=[list(range(N_CORES))],
            )
            back = pool_small.tile([1, 8], f32, name="back", tag="back")
            nc.sync.dma_start(
                out=back[:], in_=part_out[:].rearrange("a b -> (a b)")[None, :]
            )
            tot = pool_small.tile([1, 1], f32, name="tot", tag="tot")
            nc.vector.reduce_sum(out=tot[:], in_=back[:], axis=AX.X)
            lout = pool_small.tile([1, 1], f32, name="lout", tag="lout")
            nc.scalar.mul(lout[:], tot[:], 1.0 / TWO_B)
            nc.sync.dma_start(out=loss[:], in_=lout[:])

    nc.compile()
    return nc


def kernel(features_1: np.ndarray, features_2: np.ndarray) -> np.ndarray:
    from concourse.bass_utils import run_bass_kernel_spmd

    if "nc" not in _cache:
        _cache["nc"] = _build()
    nc = _cache["nc"]

    f1 = np.ascontiguousarray(np.asarray(features_1, dtype=np.float32))
    f2 = np.ascontiguousarray(np.asarray(features_2, dtype=np.float32))
    f = np.concatenate([f1, f2], axis=0)  # [2B, D]
    ftb = np.ascontiguousarray(f.T).reshape(D, N_CORES, BLK)  # [D, 8, 1024]

    ident = np.eye(128, dtype=np.float32)
    maskinv = (1.0 - ident).astype(np.float32)

    in_maps = []
    for c in range(N_CORES):
        order = [(c + j) % N_CORES for j in range(N_CORES)]
        ft_c = np.ascontiguousarray(ftb[:, order, :]).reshape(D, TWO_B)
        perm_c = np.zeros((8, 8), dtype=np.float32)
        for j in range(N_CORES):
            perm_c[(c + j) % N_CORES, j] = 1.0
        in_maps.append(
            {"ft": ft_c, "perm": perm_c, "ident": ident, "maskinv": maskinv}
        )

    res = run_bass_kernel_spmd(nc, in_maps, list(range(N_CORES)))
    out = res.results[0]["loss"]
    return np.float32(out.reshape(()))



# revision 9
# speedup vs baseline: 1.4735x; 1.4735x over previous
"""Contrastive loss (NT-Xent) on 8 Trainium2 NeuronCores.

Row-parallel over the [2B, 2B] similarity matrix: core c computes rows
[c*1024, (c+1)*1024). Inputs are passed host-transposed ([D, 2B]) with the
column blocks rotated per core so the diagonal / positive blocks land at
fixed tile indices on every core (uniform SPMD program).

Features ship as fp8(e4m3, x16) and the sim matmuls run fp8 DoubleRow
(256-deep contraction per instruction). Normalization is applied AFTER the
matmul: each core computes per-column sum-of-squares locally (squares on DVE,
ones-matmul on PE), turns them into inverse norms via exp(-0.5*ln(x)) (Ln and
Exp share one ACT table set with the logsumexp), and fuses
(psum * row_scale) * col_scale into one scalar_tensor_tensor per tile. No
mid-kernel collective: only a warmup AllGather at t=0 (absorbs channel setup)
and the final scalar AllGather for the loss psum-mean.
"""

import os
import sys

for _p in ("/opt/trn_rl_repo", "/root/.axon_site/_ro/trn_rl_repo"):
    if os.path.isdir(_p) and _p not in sys.path:
        sys.path.append(_p)

import numpy as np

B = 4096
D = 1024
TWO_B = 2 * B
TEMP = 0.07
N_CORES = 8
BLK = TWO_B // N_CORES  # 1024 rows per core
NT = TWO_B // 512  # 16 column tiles of 512
MT = BLK // 128  # 8 row tiles of 128
TT = D // 256  # 4 DoubleRow contraction steps of 256 (=2 chunks of 128)
QSCALE = 16.0  # fp8 quantization scale (cancels via rsqrt of quantized sumsq)
PREP_AHEAD = 4  # software pipeline depth (column tiles prepped ahead)

_cache = {}


def _build():
    import concourse.bass as bass  # noqa: F401
    import concourse.bacc as bacc
    import concourse.mybir as mybir
    from concourse.tile import TileContext

    f32 = mybir.dt.float32
    bf16 = mybir.dt.bfloat16
    f8 = mybir.dt.float8e4
    AF = mybir.ActivationFunctionType
    ALU = mybir.AluOpType
    AX = mybir.AxisListType
    DR = mybir.MatmulPerfMode.DoubleRow

    nc = bacc.Bacc(None, target_bir_lowering=False, debug=False)
    # row k = chunk*128 + p, chunk = 0..7; columns rotated per core
    ftq = nc.dram_tensor("ftq", [D, TWO_B], f8, kind="ExternalInput")
    ident = nc.dram_tensor("ident", [128, 128], f32, kind="ExternalInput")
    maskinv = nc.dram_tensor("maskinv", [128, 128], f32, kind="ExternalInput")
    loss = nc.dram_tensor("loss", [1, 1], f32, kind="ExternalOutput")
    debug = os.environ.get("CL_DEBUG") == "1"
    if debug:
        dbg = nc.dram_tensor("dbg", [128, 8 + 512 * 3], f32, kind="ExternalOutput")

    with TileContext(nc) as tc:
        with (
            tc.tile_pool(name="own", bufs=TT) as pool_own,
            tc.tile_pool(name="rhs", bufs=TT * (PREP_AHEAD + 2)) as pool_rhs,
            tc.tile_pool(name="sq", bufs=TT * 2) as pool_sq,
            tc.tile_pool(name="cvec", bufs=PREP_AHEAD + 2) as pool_cvec,
            tc.tile_pool(name="lnt", bufs=2) as pool_lnt,
            tc.tile_pool(name="ssb", bufs=4) as pool_ssb,
            tc.tile_pool(name="tsb", bufs=4) as pool_tsb,
            tc.tile_pool(name="exp", bufs=4) as pool_exp,
            tc.tile_pool(name="big", bufs=1) as pool_big,
            tc.tile_pool(name="small", bufs=1) as pool_small,
            tc.tile_pool(name="junk", bufs=2) as pool_junk,
            tc.tile_pool(name="psim", bufs=4, space="PSUM") as psum_sim,
            tc.tile_pool(name="pnorm", bufs=2, space="PSUM") as psum_norm,
            tc.tile_pool(name="dram", bufs=4, space="DRAM") as dram,
        ):
            warm_in = dram.tile([1, 1], f32, name="warm_in")
            warm_out = dram.tile([8, 1], f32, name="warm_out")
            part_in = dram.tile([1, 1], f32, name="part_in")
            part_out = dram.tile([8, 1], f32, name="part_out")

            # --- collective-stack warmup: absorbs one-time ncfw/channel setup
            # concurrently with the main loop ---
            warm_sb = pool_small.tile([1, 1], f32, name="warm_sb", tag="warm_sb")
            nc.vector.memset(warm_sb[:], 0.0)
            nc.sync.dma_start(out=warm_in[:], in_=warm_sb[:])
            nc.gpsimd.collective_compute(
                "AllGather",
                mybir.AluOpType.bypass,
                ins=[warm_in.opt()],
                outs=[warm_out.opt()],
                replica_groups=[list(range(N_CORES))],
            )

            # --- constants ---
            ones_f = pool_small.tile([128, 1], f32, name="ones_f", tag="ones_f")
            nc.vector.memset(ones_f[:], 1.0)
            ones_r = pool_small.tile([128, 1], bf16, name="ones_r", tag="ones_r")
            nc.vector.tensor_copy(ones_r[:], ones_f[:])
            ones1_f = pool_small.tile([1, 128], f32, name="ones1_f", tag="ones1_f")
            nc.vector.memset(ones1_f[:], 1.0)
            ones1_r = pool_small.tile([1, 128], bf16, name="ones1_r", tag="ones1_r")
            nc.vector.tensor_copy(ones1_r[:], ones1_f[:])
            ident_sb = pool_small.tile([128, 128], f32, name="ident", tag="ident")
            nc.sync.dma_start(out=ident_sb[:], in_=ident[:])
            maskinv_sb = pool_small.tile([128, 128], f32, name="maskinv", tag="maskinv")
            nc.sync.dma_start(out=maskinv_sb[:], in_=maskinv[:])

            # --- own block (lhsT for every matmul; rhs for n in {0, 1}) ---
            # own[t][p, i, col] = ftq[(2t+i)*128 + p, col]  for col in own rows
            own = []
            for t in range(TT):
                o = pool_own.tile([128, 2, BLK], f8, name="own", tag="own")
                for i in range(2):
                    nc.sync.dma_start(
                        out=o[:, i, :],
                        in_=ftq[(2 * t + i) * 128 : (2 * t + i + 1) * 128, 0:BLK],
                    )
                own.append(o)

            # --- accumulators ---
            rs_buf = pool_big.tile([128, MT * NT], f32, name="rs_buf", tag="rs_buf")
            pos_all = pool_small.tile([128, MT], f32, name="pos_all", tag="pos_all")
            nc.vector.memset(pos_all[:], 0.0)

            n_limit = int(os.environ.get("CL_NT", NT))

            rhsq = {}  # n -> list of TT rhs tiles [128, 2, 512]
            cvec = {}  # n -> [128, 512] bf16 inverse col norms (x 1/QSCALE^... cancel)
            ss_sb = {}  # n -> [1, 512] bf16 col sumsq (kept for n=0,1 row path)

            def prep(n):
                """Load + column-norm pipeline for column tile n."""
                if n < 2:
                    src = [own[t][:, :, n * 512 : (n + 1) * 512] for t in range(TT)]
                else:
                    src = []
                    for t in range(TT):
                        r = pool_rhs.tile([128, 2, 512], f8, name="rhs", tag="rhs")
                        for i in range(2):
                            nc.sync.dma_start(
                                out=r[:, i, :],
                                in_=ftq[
                                    (2 * t + i) * 128 : (2 * t + i + 1) * 128,
                                    n * 512 : (n + 1) * 512,
                                ],
                            )
                        src.append(r[:])
                    rhsq[n] = src
                # squares (DVE) then sum over partitions via ones-matmul (PE)
                ps_ss = psum_norm.tile([1, 512], f32, name="ps_ss", tag="ps_ss")
                for t in range(TT):
                    s = pool_sq.tile([128, 2, 512], bf16, name="sq", tag="sq")
                    nc.vector.tensor_mul(s[:], src[t], src[t])
                    for i in range(2):
                        nc.tensor.matmul(
                            ps_ss[:],
                            ones_r[:],
                            s[:, i, :],
                            start=(t == 0 and i == 0),
                            stop=(t == TT - 1 and i == 1),
                        )
                ssb = pool_ssb.tile([1, 512], bf16, name="ss_sb", tag="ss_sb")
                nc.vector.tensor_copy(ssb[:], ps_ss[:])
                ss_sb[n] = ssb
                # broadcast sumsq to 128 partitions, then inv-norm = exp(-ln/2)
                ps_b = psum_norm.tile([128, 512], f32, name="ps_b", tag="ps_b")
                nc.tensor.matmul(ps_b[:], ones1_r[:], ssb[:], start=True, stop=True)
                lnt = pool_lnt.tile([128, 512], f32, name="lnt", tag="lnt")
                nc.scalar.activation(lnt[:], ps_b[:], AF.Ln)
                cv = pool_cvec.tile([128, 512], bf16, name="cvec", tag="cvec")
                nc.scalar.activation(cv[:], lnt[:], AF.Exp, scale=-0.5)
                cvec[n] = cv

            # column tiles 0..3 prepped up front (0,1 are the own/diagonal block)
            for n in range(min(PREP_AHEAD, n_limit)):
                prep(n)

            # --- row scales: rrow[p, m] = (1/T) * inv-norm of row m*128+p ---
            # own rows are columns 0:1024; transpose cvec[0]/cvec[1] row 0 onto
            # partitions via rank-1 matmuls (out[:, m] = cvec_row[m*128+p] * 1)
            ones11 = pool_small.tile([1, 1], bf16, name="ones11", tag="ones11")
            nc.vector.memset(ones11[:], 1.0)
            ps_rt = psum_sim.tile([128, MT], f32, name="ps", tag="ps")
            for m in range(MT):
                nc.tensor.matmul(
                    ps_rt[:, m : m + 1],
                    cvec[m // 4][0:1, (m % 4) * 128 : (m % 4 + 1) * 128],
                    ones11[:],
                    start=True,
                    stop=True,
                )
            rrow = pool_small.tile([128, MT], f32, name="rrow", tag="rrow")
            nc.vector.tensor_scalar_mul(rrow[:], ps_rt[:], 1.0 / TEMP)
            if debug:
                dbg_rr = pool_small.tile([128, MT], f32, name="dbg_rr", tag="dbg_rr")
                nc.vector.tensor_copy(dbg_rr[:], rrow[:])
                nc.sync.dma_start(out=dbg[:, 0:8], in_=dbg_rr[:])
                dbg_cv = pool_small.tile([128, 512], f32, name="dbg_cv", tag="dbg_cv")
                nc.vector.tensor_copy(dbg_cv[:], cvec[0][:])
                nc.sync.dma_start(out=dbg[:, 8:520], in_=dbg_cv[:])

            # --- main loop: one 512-wide column tile at a time ---
            for n in range(n_limit):
                if n + PREP_AHEAD < n_limit:
                    prep(n + PREP_AHEAD)
                src = (
                    [own[t][:, :, n * 512 : (n + 1) * 512] for t in range(TT)]
                    if n < 2
                    else rhsq.pop(n)
                )
                for m in range(MT):
                    ps = psum_sim.tile([128, 512], f32, name="ps", tag="ps")
                    for t in range(TT):
                        nc.tensor.matmul(
                            ps[:],
                            own[t][:, :, m * 128 : (m + 1) * 128],
                            src[t],
                            start=(t == 0),
                            stop=(t == TT - 1),
                            perf_mode=DR,
                        )
                    # logits = (raw_dot * row_scale) * col_scale   (fused DVE)
                    tsb = pool_tsb.tile([128, 512], bf16, name="tsb", tag="tsb")
                    nc.vector.scalar_tensor_tensor(
                        tsb[:],
                        ps[:],
                        rrow[:, m : m + 1],
                        cvec[n][:],
                        ALU.mult,
                        ALU.mult,
                    )
                    if debug and n == 2 and m == 0:
                        dbg_ps = pool_small.tile(
                            [128, 512], f32, name="dbg_ps", tag="dbg_ps"
                        )
                        nc.vector.tensor_copy(dbg_ps[:], ps[:])
                        nc.sync.dma_start(out=dbg[:, 520:1032], in_=dbg_ps[:])
                        dbg_ts = pool_small.tile(
                            [128, 512], f32, name="dbg_ts", tag="dbg_ts"
                        )
                        nc.vector.tensor_copy(dbg_ts[:], tsb[:])
                        nc.sync.dma_start(out=dbg[:, 1032:1544], in_=dbg_ts[:])
                    sl = (m % 4) * 128
                    if n == 8 + m // 4:
                        # positives: diagonal of this 128x128 slab (already /T)
                        junk = pool_junk.tile([128, 128], f32, name="junk", tag="junk")
                        nc.vector.tensor_mul(junk[:], tsb[:, sl : sl + 128], ident_sb[:])
                        nc.vector.reduce_sum(
                            out=pos_all[:, m : m + 1], in_=junk[:], axis=AX.X
                        )
                    if n == m // 4:
                        # diagonal block: exp, zero the self-sim, reduce on DVE
                        e = pool_exp.tile([128, 512], f32, name="exp", tag="exp")
                        nc.scalar.activation(e[:], tsb[:], AF.Exp)
                        nc.vector.tensor_mul(
                            e[:, sl : sl + 128], e[:, sl : sl + 128], maskinv_sb[:]
                        )
                        nc.vector.reduce_sum(
                            out=rs_buf[:, m * NT + n : m * NT + n + 1],
                            in_=e[:],
                            axis=AX.X,
                        )
                    else:
                        e = pool_exp.tile([128, 512], f32, name="exp", tag="exp")
                        nc.scalar.activation(
                            e[:],
                            tsb[:],
                            AF.Exp,
                            accum_out=rs_buf[:, m * NT + n : m * NT + n + 1],
                        )

            # --- logsumexp + loss ---
            rs_all = pool_small.tile([128, MT], f32, name="rs_all", tag="rs_all")
            for m in range(MT):
                nc.vector.reduce_sum(
                    out=rs_all[:, m : m + 1],
                    in_=rs_buf[:, m * NT : m * NT + n_limit],
                    axis=AX.X,
                )
            lse = pool_small.tile([128, MT], f32, name="lse", tag="lse")
            nc.scalar.activation(lse[:], rs_all[:], AF.Ln)
            diff = pool_small.tile([128, MT], f32, name="diff", tag="diff")
            nc.vector.tensor_sub(diff[:], lse[:], pos_all[:])
            dsum = pool_small.tile([128, 1], f32, name="dsum", tag="dsum")
            nc.vector.reduce_sum(out=dsum[:], in_=diff[:], axis=AX.X)
            pf = psum_sim.tile([128, 512], f32, name="ps", tag="ps")
            nc.tensor.matmul(pf[0:1, 0:1], dsum[:], ones_f[:], start=True, stop=True)
            part_sb = pool_small.tile([1, 1], f32, name="part_sb", tag="part_sb")
            nc.vector.tensor_copy(part_sb[:], pf[0:1, 0:1])
            nc.sync.dma_start(out=part_in[:], in_=part_sb[:])
            nc.gpsimd.collective_compute(
                "AllGather",
                mybir.AluOpType.bypass,
                ins=[part_in.opt()],
                outs=[part_out.opt()],
                replica_groups=[list(range(N_CORES))],
            )
            back = pool_small.tile([1, 8], f32, name="back", tag="back")
            nc.sync.dma_start(
                out=back[:], in_=part_out[:].rearrange("a b -> (a b)")[None, :]
            )
            tot = pool_small.tile([1, 1], f32, name="tot", tag="tot")
            nc.vector.reduce_sum(out=tot[:], in_=back[:], axis=AX.X)
            lout = pool_small.tile([1, 1], f32, name="lout", tag="lout")
            nc.scalar.mul(lout[:], tot[:], 1.0 / TWO_B)
            nc.sync.dma_start(out=loss[:], in_=lout[:])

    nc.compile()
    return nc


def make_in_maps(features_1: np.ndarray, features_2: np.ndarray):
    import ml_dtypes

    f1 = np.asarray(features_1, dtype=np.float32)
    f2 = np.asarray(features_2, dtype=np.float32)
    f = np.concatenate([f1, f2], axis=0)  # [2B, D]
    ftb = np.ascontiguousarray(f.T).reshape(D, N_CORES, BLK)  # [D, 8, 1024]

    ident = np.eye(128, dtype=np.float32)
    maskinv = (1.0 - ident).astype(np.float32)

    in_maps = []
    for c in range(N_CORES):
        order = [(c + j) % N_CORES for j in range(N_CORES)]
        ft_c = np.ascontiguousarray(ftb[:, order, :]).reshape(D, TWO_B)
        ftq_c = np.clip(ft_c * QSCALE, -240.0, 240.0).astype(ml_dtypes.float8_e4m3)
        in_maps.append({"ftq": ftq_c, "ident": ident, "maskinv": maskinv})
    return in_maps


def kernel(features_1: np.ndarray, features_2: np.ndarray) -> np.ndarray:
    from concourse.bass_utils import run_bass_kernel_spmd

    if "nc" not in _cache:
        _cache["nc"] = _build()
    nc = _cache["nc"]

    in_maps = make_in_maps(features_1, features_2)
    res = run_bass_kernel_spmd(nc, in_maps, list(range(N_CORES)))
    out = res.results[0]["loss"]
    return np.float32(out.reshape(()))
